# revision 83
# baseline (speedup 1.0000x reference)
"""DeformConvBlock Trainium2 kernel (data-parallel over batch across 8 cores).

Per-core (1 image, C=128, O=128, H=W=80, 3x3):
  1. offset = conv3x3(x, w_off) + b_off            (PE bf16 im2col GEMM)
  2. bilinear deform sampling via affine-basis identity:
       sample = P0[q] + dy*P1[q] + dx*P2[q] + dy*dx*P3[q],
     q = (floor(py), floor(px)) clamped to an 84x84 window of the 8-padded
     image; P0..P3 = x and its v/h/cross shifted differences.
  3. dma_gather 57.6K rows (1KB each) from DRAM [q, 4*C] bf16, row =
     [P0|P2|P1|P3] so one wide scalar_tensor_tensor does both dy-lerps.
  4. v-transpose as PE matmuls v.T @ I accumulated in fp32 PSUM; the
     dx-combine rides the accumulation (dx-scaled term via ACT scale);
     batched ACT copies to SBUF; bf16 GEMM with w; + bias -> out.
"""

import contextlib
import numpy as np
import ml_dtypes

import concourse.bass as bass
import concourse.tile as tile
from concourse import bacc, mybir
from concourse import bass_utils

F32 = mybir.dt.float32
BF16 = mybir.dt.bfloat16
I16 = mybir.dt.int16
I32 = mybir.dt.int32
A = mybir.AluOpType
AF = mybir.ActivationFunctionType

N, C, O, H, W = 8, 128, 128, 80, 80
K = 9
WP = 96                   # padded image pitch (PAD=8 frame)
PAD = 8
PW = 84                   # p4 window size: padded rows/cols [6, 90)
W0 = 6                    # window origin in the padded frame
QP4 = PW * PW             # 7056
HWi = H * W               # 6400
NT = HWi // 128           # 50 pixel tiles
NTT = NT * K              # 450 (tap, tile) pairs
NJ = NTT * 128            # 57600 gather rows
CLO, CHI = float(W0), float(W0 + PW - 2)   # coord clamp [6, 88]


def _mrep_np():
    # column u*128+c has a single 1 at row u*16 + (c % 16):
    # matmul M_u.T @ fidx replicates partition group [u*16,(u+1)*16) 8x.
    m = np.zeros((128, 8 * 128), np.float32)
    for u in range(8):
        for c in range(128):
            m[u * 16 + (c % 16), u * 128 + c] = 1.0
    return m


def _baseT_np():
    p = np.arange(HWi)
    py, px = p // W, p % W
    kh = np.arange(K) // 3 - 1
    kw = np.arange(K) % 3 - 1
    base = np.zeros((HWi, 18), np.float32)
    base[:, 0::2] = py[:, None] + kh[None, :] + PAD
    base[:, 1::2] = px[:, None] + kw[None, :] + PAD
    return np.ascontiguousarray(
        base.reshape(NT, 128, 18).transpose(1, 0, 2).reshape(128, NT * 18))


def build_kernel(num_devices=N, debug=False):
    nc = bacc.Bacc("TRN2", target_bir_lowering=False, debug=False,
                   num_devices=num_devices, dynamic_dma_scratch_size=49152)

    x_in = nc.dram_tensor("x", [C, HWi], F32, kind="ExternalInput").ap()
    w_off_t = nc.dram_tensor("w_off_t", [C, K * 18], BF16, kind="ExternalInput").ap()
    w_t = nc.dram_tensor("w_t", [C, K * O], BF16, kind="ExternalInput").ap()
    b_in = nc.dram_tensor("b", [O, 1], F32, kind="ExternalInput").ap()
    boff_in = nc.dram_tensor("b_off", [18, 1], F32, kind="ExternalInput").ap()
    baseT_in = nc.inline_tensor(_baseT_np(), "baseT").ap()
    identb_in = nc.inline_tensor(np.eye(128, dtype=ml_dtypes.bfloat16), "identb").ap()
    identf_in = nc.inline_tensor(np.eye(18, dtype=np.float32), "identf").ap()
    mrep_in = nc.inline_tensor(_mrep_np(), "mrep").ap()

    y_out = nc.dram_tensor("y", [O, HWi], F32, kind="ExternalOutput").ap()

    p4_dram = nc.dram_tensor("p4_dram", [QP4, 4 * C], BF16, kind="Internal").ap()

    with tile.TileContext(nc) as tc:
        with contextlib.ExitStack() as ctx:
            _body(ctx, tc, nc, x_in, w_off_t, w_t, b_in, boff_in, baseT_in,
                  identb_in, identf_in, mrep_in, y_out, p4_dram)
    nc.compile()
    return nc


def _body(ctx, tc, nc, x_in, w_off_t, w_t, b_in, boff_in, baseT_in,
          identb_in, identf_in, mrep_in, y_out, p4_dram):
    const = ctx.enter_context(tc.tile_pool(name="const", bufs=1))
    pers = ctx.enter_context(tc.tile_pool(name="pers", bufs=1))

    # ---- constants ----
    identb = const.tile([128, 128], BF16)
    nc.sync.dma_start(identb[:], identb_in)
    identf = const.tile([18, 18], F32)
    nc.sync.dma_start(identf[:], identf_in)
    bias = const.tile([O, 1], F32)
    nc.sync.dma_start(bias[:], b_in)
    boff = const.tile([18, 1], F32)
    nc.sync.dma_start(boff[:], boff_in)
    baseT = const.tile([C, NT * 18], F32)
    nc.sync.dma_start(baseT[:], baseT_in)
    woff = const.tile([C, K * 18], BF16)
    nc.sync.dma_start(woff[:], w_off_t)
    wmat = const.tile([C, K * O], BF16)
    nc.sync.dma_start(wmat[:], w_t)
    mrep = const.tile([128, 8 * 128], F32)
    nc.sync.dma_start(mrep[:], mrep_in)

    # ---- persistent SBUF (live through phase C) ----
    dd = pers.tile([128, NT * 18], F32)      # fractional dy/dx, col 2T/2T+1
    idxW = pers.tile([128, NJ // 16], I16)

    # ================= phases A/B: conv, maps, planes, p4 =================
    with tc.tile_pool(name="ph1", bufs=1) as ph1, \
         tc.tile_pool(name="ph1s", bufs=4) as ph1s, \
         tc.tile_pool(name="mapsc", bufs=1) as mp, \
         tc.tile_pool(name="ps_off", bufs=2, space="PSUM") as ps_off, \
         tc.tile_pool(name="ps_map", bufs=2, space="PSUM") as ps_map, \
         tc.tile_pool(name="ps_pw", bufs=1, space="PSUM") as ps_pw, \
         tc.tile_pool(name="ps_pk", bufs=3, space="PSUM") as ps_pk:
        # dense x load with f32->bf16 cast on the DMA (SWDGE)
        xbd = ph1.tile([C, HWi], BF16)
        nc.gpsimd.dma_start(xbd[:], x_in)

        # padded bf16 image: zero ring around the used window, interior copy
        xbp = ph1.tile([C, WP * WP], BF16)
        xb3 = xbp[:].rearrange("c (h w) -> c h w", h=WP)
        nc.vector.memset(xb3[:, 6:8, :], 0.0)       # top rows 6,7 (full width)
        nc.vector.memset(xb3[:, 88:92, :], 0.0)     # bottom rows 88-91 (full width)
        nc.vector.memset(xb3[:, 8:88, 0:8], 0.0)    # left cols 0-7
        nc.vector.memset(xb3[:, 8:88, 88:96], 0.0)  # right cols 88-95
        nc.vector.tensor_copy(xb3[:, PAD:PAD + H, PAD:PAD + W],
                              xbd[:].rearrange("c (h w) -> c h w", h=H))

        # ---- p4 row emitter: 2 rows per PSUM bank, 4-row staged stores ----
        planes = []
        pst = {"pk": None, "stg": None}

        def emit_p4_rows(y_lo, y_hi):
            for yi in range(y_lo, y_hi):
                qp0 = (W0 + yi) * WP + W0
                if yi % 2 == 0:
                    pst["pk"] = ps_pk.tile([PW, 2, 4, 128], BF16, tag="pk", name="pk")
                pk = pst["pk"]
                for pi, pl in enumerate(planes):
                    nc.tensor.transpose(pk[:, yi % 2, pi, :],
                                        pl[:, qp0:qp0 + PW], identb[:])
                if yi % 2 == 1:
                    if yi % 4 == 1:
                        pst["stg"] = ph1s.tile([PW, 4, 4 * C], BF16, tag="stg", name="stg")
                    stg = pst["stg"]
                    dst = stg[:, (yi % 4) - 1:(yi % 4) + 1, :].rearrange(
                        "p r f -> p (r f)")
                    src = pk[:].rearrange("p r f c -> p (r f c)")
                    if yi % 4 == 1:
                        nc.scalar.copy(dst, src)
                    else:
                        nc.vector.tensor_copy(dst, src)
                if yi % 4 == 3:
                    r0 = yi - 3
                    deng = nc.sync if yi % 8 == 3 else nc.gpsimd
                    deng.dma_start(
                        p4_dram[r0 * PW:(r0 + 4) * PW, :].rearrange(
                            "(r p) f -> p r f", r=4), pst["stg"][:])

        # offset conv (bf16) chunks interleaved with p4 row building
        off_sb = ph1.tile([18, HWi], F32)
        CH = 6
        for ci, yc in enumerate(range(0, H, CH)):
            rows = min(CH, H - yc)
            po = ps_off.tile([18, CH * W], F32, tag="po")
            for k in range(K):
                kh, kw = divmod(k, 3)
                rhs = xb3[:, (yc + kh - 1 + PAD):(yc + kh - 1 + PAD) + rows,
                          (kw - 1 + PAD):(kw - 1 + PAD) + W]
                nc.tensor.matmul(po[:, :rows * W],
                                 woff[:, k * 18:(k + 1) * 18], rhs,
                                 start=(k == 0), stop=(k == K - 1))
            nc.vector.tensor_scalar_add(off_sb[:, yc * W:(yc + rows) * W],
                                        po[:, :rows * W], boff[:])

        # ---- maps (batched over all 50 tiles) ----
        offT = mp.tile([128, NT * 18], F32)
        for half in range(2):
            pt = ps_map.tile([128, 25 * 18], F32, tag="mapT")
            for i in range(25):
                t = half * 25 + i
                nc.tensor.transpose(pt[:, i * 18:(i + 1) * 18],
                                    off_sb[:, t * 128:(t + 1) * 128], identf[:])
            nc.scalar.copy(offT[:, half * 450:(half + 1) * 450], pt[:])
        q = mp.tile([128, NT * 18], F32)
        nc.vector.tensor_tensor(q[:], offT[:], baseT[:], op=A.add)
        nc.vector.tensor_scalar_min(q[:], q[:], CHI)
        nc.vector.tensor_scalar_max(q[:], q[:], CLO)
        qi = mp.tile([128, NT * 18], I32)
        nc.vector.tensor_copy(qi[:], q[:])            # rne
        qr = mp.tile([128, NT * 18], F32)
        nc.vector.tensor_copy(qr[:], qi[:])
        m = mp.tile([128, NT * 18], F32)
        nc.vector.tensor_tensor(m[:], qr[:], q[:], op=A.is_gt)
        flp = mp.tile([128, NT * 18], F32)            # floor - 6 (window coords)
        nc.vector.scalar_tensor_tensor(flp[:], qr[:], CLO, m[:],
                                       op0=A.subtract, op1=A.subtract)
        nc.vector.scalar_tensor_tensor(dd[:], q[:], CLO, flp[:],
                                       op0=A.subtract, op1=A.subtract)
        fl4 = flp[:].rearrange("p (t k two) -> p t k two", k=K, two=2)
        fidx = mp.tile([128, NTT], F32)
        fidx3 = fidx[:].rearrange("p (t k) -> p t k", k=K)
        nc.vector.scalar_tensor_tensor(fidx3, fl4[:, :, :, 0], float(PW),
                                       fl4[:, :, :, 1], op0=A.mult, op1=A.add)

        # idx wrap via PE partition-fold: idxW[16g+r, 8T+u] = fidx[u*16+r, T]
        idxW3 = idxW[:].rearrange("p (t u) -> p t u", u=8)
        for u in range(8):
            pw = ps_pw.tile([128, NTT], F32, tag="pw")
            nc.tensor.matmul(pw[:], mrep[:, u * 128:(u + 1) * 128], fidx[:],
                             start=True, stop=True)
            nc.vector.tensor_copy(idxW3[:, :, u], pw[:])

        # ---- difference planes over the window rows (2 chunks each) ----
        d1 = ph1.tile([C, WP * WP], BF16)
        d2 = ph1.tile([C, WP * WP], BF16)
        d3 = ph1.tile([C, WP * WP], BF16)
        planes.extend([xbp, d2, d1, d3])  # p4 row = [P0|P2|P1|P3]
        for (ra, rb) in ((6, 48), (48, 90)):
            a, b = ra * WP, rb * WP
            nc.vector.tensor_tensor(d1[:, a:b], xbp[:, a + WP:b + WP],
                                    xbp[:, a:b], op=A.subtract)
        for (ra, rb) in ((6, 49), (49, 91)):
            a, b = ra * WP, rb * WP
            nc.vector.tensor_tensor(d2[:, a:b], xbp[:, a + 1:b + 1],
                                    xbp[:, a:b], op=A.subtract)
        for (ra, rb) in ((6, 48), (48, 90)):
            a, b = ra * WP, rb * WP
            nc.vector.tensor_tensor(d3[:, a:b], d2[:, a + WP:b + WP],
                                    d2[:, a:b], op=A.subtract)

        emit_p4_rows(0, PW)

    # ================= phase C: gather + combine + GEMM =================
    TCH = 1
    with tc.tile_pool(name="gpool", bufs=2, side="right") as gpool, \
         tc.tile_pool(name="vpool", bufs=4) as vpool, \
         tc.tile_pool(name="vsbp", bufs=3) as vsbp, \
         tc.tile_pool(name="opool", bufs=2) as opool, \
         tc.tile_pool(name="ps_out", bufs=2, space="PSUM") as ps_out, \
         tc.tile_pool(name="ps_vA", bufs=2, space="PSUM") as ps_vA, \
         tc.tile_pool(name="ps_vB", bufs=2, space="PSUM") as ps_vB:
        def issue_gather(tc0):
            nt = min(TCH, NT - tc0)
            nidx = nt * K * 128
            gt = gpool.tile([128, TCH * K, 4 * C], BF16, tag="gather")
            c0 = tc0 * K * 8
            nc.gpsimd.dma_gather(gt[:, :nt * K, :], p4_dram,
                                 idxW[:, c0:c0 + nidx // 16],
                                 num_idxs=nidx, num_idxs_reg=nidx, elem_size=4 * C,
                                 single_packet=False)
            return gt

        ot = None
        gt = issue_gather(0)
        for tc0 in range(0, NT, TCH):
            nt = min(TCH, NT - tc0)
            if tc0 + TCH < NT:
                gt_next = issue_gather(tc0 + TCH)
            for ti in range(nt):
                t = tc0 + ti
                vA0 = ps_vA.tile([C, 4, 128], F32, tag="vA0")
                vA1 = ps_vA.tile([C, 4, 128], F32, tag="vA1")
                vB = ps_vB.tile([C, 128], F32, tag="vB")
                vsb = vsbp.tile([C, K, 128], BF16, tag="vsb")
                for k in range(K):
                    T = t * K + k
                    g = gt[:, ti * K + k, :]
                    dy = dd[:, 2 * T:2 * T + 1]
                    dx = dd[:, 2 * T + 1:2 * T + 2]
                    # row = [P0|P2|P1|P3]: one wide STT does both dy-lerps
                    s12 = vpool.tile([128, 2 * C], BF16, tag="s12")
                    nc.vector.scalar_tensor_tensor(
                        s12[:], g[:, 2 * C:4 * C], dy, g[:, 0:2 * C],
                        op0=A.mult, op1=A.add)
                    vt = (vA0[:, k, :] if k < 4 else
                          vA1[:, k - 4, :] if k < 8 else vB[:])
                    if k < 6:
                        # dx-scale on ACT; v transpose-sum on PE:
                        #   vT = s1.T @ I + (dx*s2).T @ I
                        u2 = vpool.tile([128, C], BF16, tag="u2")
                        nc.scalar.activation(u2[:], s12[:, C:2 * C], AF.Identity,
                                             scale=dx)
                        nc.tensor.matmul(vt, s12[:, 0:C], identb[:],
                                         start=True, stop=False)
                        nc.tensor.matmul(vt, u2[:], identb[:],
                                         start=False, stop=True)
                    else:
                        v = vpool.tile([128, C], BF16, tag="v")
                        nc.vector.scalar_tensor_tensor(
                            v[:], s12[:, C:2 * C], dx, s12[:, 0:C],
                            op0=A.mult, op1=A.add)
                        nc.tensor.matmul(vt, v[:], identb[:],
                                         start=True, stop=True)
                nc.scalar.copy(vsb[:, 0:4, :].rearrange("c k p -> c (k p)"),
                               vA0[:].rearrange("c k p -> c (k p)"))
                nc.scalar.copy(vsb[:, 4:8, :].rearrange("c k p -> c (k p)"),
                               vA1[:].rearrange("c k p -> c (k p)"))
                nc.scalar.copy(vsb[:, 8, :], vB[:])
                out_ps = ps_out.tile([O, 128], F32, tag="ops")
                for k in range(K):
                    nc.tensor.matmul(out_ps[:], wmat[:, k * O:(k + 1) * O],
                                     vsb[:, k, :], start=(k == 0), stop=(k == K - 1))
                if t % 4 == 0:
                    ot = opool.tile([O, 512], F32, tag="ot")
                nc.scalar.activation(ot[:, (t % 4) * 128:(t % 4 + 1) * 128],
                                     out_ps[:], AF.Identity, bias=bias[:])
                if t % 4 == 3 or t == NT - 1:
                    j0 = (t - t % 4) * 128
                    nc.sync.dma_start(y_out[:, j0:(t + 1) * 128],
                                      ot[:, :(t % 4 + 1) * 128])
            gt = gt_next if tc0 + TCH < NT else gt


# ================= host side =================

def _prep_inputs(x, w_off, b_off, w, b):
    # [C, K*18]: col k*18+e = w_off[e, c, k]
    wofft = np.ascontiguousarray(
        w_off.reshape(18, C, K).transpose(1, 2, 0).reshape(C, K * 18)).astype(
            ml_dtypes.bfloat16)
    wt = np.ascontiguousarray(
        w.reshape(O, C, K).transpose(1, 2, 0).reshape(C, K * O)).astype(ml_dtypes.bfloat16)
    shared = {
        "w_off_t": wofft,
        "w_t": wt,
        "b": np.ascontiguousarray(b.reshape(O, 1)).astype(np.float32),
        "b_off": np.ascontiguousarray(b_off.reshape(18, 1)).astype(np.float32),
    }
    return [dict(shared, x=np.ascontiguousarray(x[n].reshape(C, HWi)).astype(np.float32))
            for n in range(x.shape[0])]


_CACHED = {}


def _get_nc(num_devices=N, debug=False):
    key = (num_devices, debug)
    if key not in _CACHED:
        _CACHED[key] = build_kernel(num_devices=num_devices, debug=debug)
    return _CACHED[key]


def kernel(x, w_off, b_off, w, b):
    x = np.asarray(x, np.float32)
    nc = _get_nc()
    core_ins = _prep_inputs(x, np.asarray(w_off, np.float32),
                            np.asarray(b_off, np.float32),
                            np.asarray(w, np.float32), np.asarray(b, np.float32))
    res = bass_utils.run_bass_kernel_spmd(nc, core_ins, core_ids=list(range(N)))
    return np.stack([res.results[n]["y"].reshape(O, H, W) for n in range(N)]).astype(np.float32)
